# revision 5
# baseline (speedup 1.0000x reference)
"""Masked-loss kernel for nn_MLoss_9715216024200 on 8 Trainium2 NeuronCores.

loss = sum(where(y[...,0]>0.5, (y-x)^2 - a*x^2, 0)) + a*sum(x[...,0]^2)
with x,y f32 (256, 10647, 5); output is a f32 scalar.

Sharding: flatten both tensors to cells (5 contiguous f32 each), pad with
256 zero-cells (neutral: y0=0 -> mask 0, x=0 -> no bg term), reshape to
(8 cores, 128 partitions, 2662 cells).  Each core streams its 13 MiB at
the 360 GB/s DMA roofline (the stream is gapless); the design minimizes
the HEAD (fixed ~2us) and the TAIL (what happens after the last byte).

Per tile (cells ~134 shrinking to 8 at the end):
  DVE :  m5 = bf16(y0>0.5) broadcast to 5 features  (2x mode, 0.52ns/elem)
         d  = y - x (f32 1x, bf16 out)
         dm = d * m5 (bf16 2x) -> group dmx buffer [dm|xs0] slices
  Pool:  xs0 = sqrt(a)*x0 -> dmx tail slice;  xm = x*m5 -> group xm buffer
Reductions per GROUP of tiles (m*v^2 == (m*v)^2 since m is 0/1):
  acc[2g]   = sum(dmx^2) = sum((m*d)^2) + a*sum(x0^2)
  acc[2g+1] = sum(xm^2)            (host multiplies by -a)
Main groups use ACT Square+accum (ACT is otherwise idle); the small
suffix groups use DVE scalar_tensor_tensor accumulates so the endgame
after the final DMA semaphore is a short single-engine chain.

The tail tiles shrink gradually (96..8) so DVE's ~900ns semaphore lag
drains before the last tile; the final tile's whole chain (m5,d,dm,xm,
2 TTRs) runs on DVE back-to-back.  Host combines in f64:
total = sum(acc[even]) - a*sum(acc[odd]).
"""
import sys

for _p in ('/opt/trn_rl_repo',):
    if _p in sys.path:
        sys.path.remove(_p)
    sys.path.insert(0, _p)

import numpy as np

B, C, F = 256, 10647, 5
THRESH = 0.5
ALPHA = 0.1
N_CORES = 8
P = 128
CELLS = B * C                      # 2,725,632
CELLS_PER_PART = 2662              # 8*128*2662 = 2,725,888
PAD_CELLS = N_CORES * P * CELLS_PER_PART - CELLS   # 256
FD = CELLS_PER_PART * F            # 13310 elems per partition per core

# --- schedule configuration -------------------------------------------
SUFFIX = [96, 72, 56, 40, 28, 16, 8]          # shrinking tail tiles
_main_total = CELLS_PER_PART - sum(SUFFIX)    # 2346
_n_main = 18
_base = _main_total // _n_main
_rem = _main_total - _base * _n_main
MAIN = [_base + (1 if i < _rem else 0) for i in range(_n_main)]
TILE_SIZES = MAIN + SUFFIX
assert sum(TILE_SIZES) == CELLS_PER_PART

# reduction groups over consecutive tiles; engine 'act' or 'dve'
GROUP_OF = [3] * 6 + [2, 2, 2, 1]             # main 6x3, suffix {96,72},{56,40},{28,16},{8}
GROUP_ENG = ['act'] * 6 + ['act', 'act', 'dve', 'dve']
assert sum(GROUP_OF) == len(TILE_SIZES)
N_GROUPS = len(GROUP_OF)
_tile_group = [(gi, k) for gi, gn in enumerate(GROUP_OF) for k in range(gn)]
XM_ON_DVE = {len(TILE_SIZES) - 1}   # last tile: keep whole chain on DVE
DEFER_LAST = 1                      # defer the last k dve-group TTR pairs
BUFS = (12, 12, 8, 4)

_compiled = None


def _build():
    from contextlib import ExitStack
    import concourse.tile as tile
    from concourse import bacc, mybir

    sqa = float(np.sqrt(ALPHA))

    nc = bacc.Bacc("TRN2", target_bir_lowering=False, debug=False,
                   enable_asserts=True, num_devices=N_CORES)
    x_d = nc.dram_tensor("x", [P, FD], mybir.dt.float32, kind="ExternalInput").ap()
    y_d = nc.dram_tensor("y", [P, FD], mybir.dt.float32, kind="ExternalInput").ap()
    o_d = nc.dram_tensor("o", [P, 2 * N_GROUPS], mybir.dt.float32,
                         kind="ExternalOutput").ap()

    f32 = mybir.dt.float32
    bf16 = mybir.dt.bfloat16
    Sq = mybir.ActivationFunctionType.Square
    Alu = mybir.AluOpType

    with tile.TileContext(nc) as tc, ExitStack() as ctx:
        xp = ctx.enter_context(tc.tile_pool(name="x", bufs=BUFS[0]))
        yp = ctx.enter_context(tc.tile_pool(name="y", bufs=BUFS[1]))
        wp = ctx.enter_context(tc.tile_pool(name="work", bufs=BUFS[2]))
        sp = ctx.enter_context(tc.tile_pool(name="scratch", bufs=BUFS[3]))
        ap_ = ctx.enter_context(tc.tile_pool(name="acc", bufs=1))

        acc = ap_.tile([P, 2 * N_GROUPS], f32)

        deferred = []
        off = 0
        gdmx = gxm = None
        gdoff = gxoff = 0
        for t, cells in enumerate(TILE_SIZES):
            fd = cells * F
            g, k_in_g = _tile_group[t]
            gn = GROUP_OF[g]
            gcells = sum(TILE_SIZES[t - k_in_g:t - k_in_g + gn])
            xt = xp.tile([P, fd], f32, tag="xt")
            yt = yp.tile([P, fd], f32, tag="yt")
            sl = slice(off, off + fd)
            off += fd
            # two issue queues: y on SP, x on ACT -- one queue's ~1300ns
            # SEQ+HWDGE prep per pair can't keep up with the stream
            nc.sync.dma_start(yt[:], y_d[:, sl])
            nc.scalar.dma_start(xt[:], x_d[:, sl])

            if k_in_g == 0:
                # group buffers: dmx = [dm(t0)|xs0(t0)|dm(t1)|xs0(t1)|...]
                gdmx = wp.tile([P, (gcells * F) + gcells], bf16, tag="dmx")
                gxm = wp.tile([P, gcells * F], bf16, tag="xmg")
                gdoff = gxoff = 0

            # DVE: mask replicated to all 5 features (2x mode)
            m5 = wp.tile([P, fd], bf16, tag="m5")
            y0b = yt[:, 0::F].unsqueeze(2).broadcast_to((P, cells, F))
            nc.vector.tensor_scalar(
                m5[:].rearrange("p (k f) -> p k f", f=F), y0b,
                THRESH, None, op0=Alu.is_gt)

            # Pool: xs0 = sqrt(a)*x0 into this tile's dmx tail slice
            nc.gpsimd.tensor_scalar(
                gdmx[:, gdoff + fd:gdoff + fd + cells], xt[:, 0::F],
                sqa, None, op0=Alu.mult)

            # DVE: d = y - x (bf16 out), dm = d*m5 (bf16 2x)
            dt_ = wp.tile([P, fd], bf16, tag="d")
            nc.vector.tensor_tensor(dt_[:], yt[:], xt[:], op=Alu.subtract)
            nc.vector.tensor_tensor(gdmx[:, gdoff:gdoff + fd], dt_[:], m5[:],
                                    op=Alu.mult)
            xm_eng = nc.vector if t in XM_ON_DVE else nc.gpsimd
            xm_eng.tensor_tensor(gxm[:, gxoff:gxoff + fd], xt[:], m5[:],
                                 op=Alu.mult)
            gdoff += fd + cells
            gxoff += fd

            if k_in_g == gn - 1:
                n_dve_after = sum(1 for gg in range(g, N_GROUPS)
                                  if GROUP_ENG[gg] == 'dve')
                if GROUP_ENG[g] == 'act':
                    sq = sp.tile([P, gcells * F + gcells], bf16, tag="sq")
                    nc.scalar.activation(sq[:], gdmx[:], Sq,
                                         accum_out=acc[:, 2 * g:2 * g + 1])
                    sq2 = sp.tile([P, gcells * F], bf16, tag="sq2")
                    nc.scalar.activation(sq2[:], gxm[:], Sq,
                                         accum_out=acc[:, 2 * g + 1:2 * g + 2])
                elif n_dve_after <= DEFER_LAST:
                    deferred.append((g, gdmx, gxm, gcells))
                else:
                    _emit_ttr(nc, sp, acc, g, gdmx, gxm, gcells, bf16, Alu)

        for (g, gdmx, gxm, gcells) in deferred:
            _emit_ttr(nc, sp, acc, g, gdmx, gxm, gcells, bf16, Alu)

        nc.sync.dma_start(o_d[:], acc[:])

    nc.compile()
    return nc


def _emit_ttr(nc, sp, acc, g, gdmx, gxm, gcells, bf16, Alu):
    # DVE fused multiply + row-sum accumulate: acc += sum(in0*in1) per row
    scr = sp.tile([P, gcells * 6], bf16, tag="ttr1")
    nc.vector.scalar_tensor_tensor(
        scr[:], gdmx[:], 1.0, gdmx[:], op0=Alu.mult, op1=Alu.mult,
        accum_out=acc[:, 2 * g:2 * g + 1])
    scr2 = sp.tile([P, gcells * 5], bf16, tag="ttr2")
    nc.vector.scalar_tensor_tensor(
        scr2[:], gxm[:], 1.0, gxm[:], op0=Alu.mult, op1=Alu.mult,
        accum_out=acc[:, 2 * g + 1:2 * g + 2])


def _shard(a: np.ndarray) -> list[np.ndarray]:
    flat = a.reshape(-1)
    pad = np.zeros(PAD_CELLS * F, dtype=a.dtype)
    flat = np.concatenate([flat, pad])
    per_core = flat.reshape(N_CORES, P, FD)
    return [np.ascontiguousarray(per_core[i]) for i in range(N_CORES)]


def kernel(x: np.ndarray, y: np.ndarray) -> np.ndarray:
    global _compiled
    if _compiled is None:
        _compiled = _build()
    nc = _compiled

    from concourse.bass_utils import run_bass_kernel_spmd

    xs = _shard(np.asarray(x, dtype=np.float32))
    ys = _shard(np.asarray(y, dtype=np.float32))
    in_maps = [{"x": xs[i], "y": ys[i]} for i in range(N_CORES)]
    res = run_bass_kernel_spmd(nc, in_maps, core_ids=list(range(N_CORES)))

    total = np.float64(0.0)
    for r in res.results:
        o = r["o"].astype(np.float64).reshape(P, 2 * N_GROUPS)
        total += o[:, 0::2].sum()
        total -= ALPHA * o[:, 1::2].sum()
    return np.float32(total)


# revision 6
# speedup vs baseline: 1.0584x; 1.0584x over previous
"""Masked-loss kernel for nn_MLoss_9715216024200 on 8 Trainium2 NeuronCores.

loss = sum(where(y[...,0]>0.5, (y-x)^2 - a*x^2, 0)) + a*sum(x[...,0]^2)
with x,y f32 (256, 10647, 5); output is a f32 scalar.

Sharding: flatten both tensors to cells (5 contiguous f32 each), pad with
256 zero-cells (neutral: y0=0 -> mask 0, x=0 -> no bg term), reshape to
(8 cores, 128 partitions, 2662 cells).  Each core streams its 13 MiB at
the 360 GB/s DMA roofline (the stream is gapless); the design minimizes
the HEAD (fixed ~2us) and the TAIL (what happens after the last byte).

Per tile (cells ~134 shrinking to 8 at the end):
  DVE :  m5 = bf16(y0>0.5) broadcast to 5 features  (2x mode, 0.52ns/elem)
         d  = y - x (f32 1x, bf16 out)
         dm = d * m5 (bf16 2x) -> group dmx buffer [dm|xs0] slices
  Pool:  xs0 = sqrt(a)*x0 -> dmx tail slice;  xm = x*m5 -> group xm buffer
Reductions per GROUP of tiles (m*v^2 == (m*v)^2 since m is 0/1):
  acc[2g]   = sum(dmx^2) = sum((m*d)^2) + a*sum(x0^2)
  acc[2g+1] = sum(xm^2)            (host multiplies by -a)
Main groups use ACT Square+accum (ACT is otherwise idle); the small
suffix groups use DVE scalar_tensor_tensor accumulates so the endgame
after the final DMA semaphore is a short single-engine chain.

The tail tiles shrink gradually (96..8) so DVE's ~900ns semaphore lag
drains before the last tile; the final tile's whole chain (m5,d,dm,xm,
2 TTRs) runs on DVE back-to-back.  Host combines in f64:
total = sum(acc[even]) - a*sum(acc[odd]).
"""
import sys

for _p in ('/opt/trn_rl_repo',):
    if _p in sys.path:
        sys.path.remove(_p)
    sys.path.insert(0, _p)

import numpy as np

B, C, F = 256, 10647, 5
THRESH = 0.5
ALPHA = 0.1
N_CORES = 8
P = 128
CELLS = B * C                      # 2,725,632
CELLS_PER_PART = 2662              # 8*128*2662 = 2,725,888
PAD_CELLS = N_CORES * P * CELLS_PER_PART - CELLS   # 256
FD = CELLS_PER_PART * F            # 13310 elems per partition per core

# --- schedule configuration -------------------------------------------
SUFFIX = [96, 72, 56, 40, 28, 16, 8]          # shrinking tail tiles
_main_total = CELLS_PER_PART - sum(SUFFIX)    # 2346
_n_main = 18
_base = _main_total // _n_main
_rem = _main_total - _base * _n_main
MAIN = [_base + (1 if i < _rem else 0) for i in range(_n_main)]
TILE_SIZES = MAIN + SUFFIX
assert sum(TILE_SIZES) == CELLS_PER_PART

# reduction groups over consecutive tiles; engine 'act' or 'dve'
GROUP_OF = [3] * 6 + [2, 2, 2, 1]             # main 6x3, suffix {96,72},{56,40},{28,16},{8}
GROUP_ENG = ['act'] * 6 + ['act', 'act', 'dve', 'dve']
assert sum(GROUP_OF) == len(TILE_SIZES)
N_GROUPS = len(GROUP_OF)
_tile_group = [(gi, k) for gi, gn in enumerate(GROUP_OF) for k in range(gn)]
XM_ON_DVE = {len(TILE_SIZES) - 1}   # last tile: keep whole chain on DVE
DEFER_LAST = 1                      # defer the last k dve-group TTR pairs
BUFS = (12, 12, 8, 4)

_compiled = None


def _build():
    from contextlib import ExitStack
    import concourse.tile as tile
    from concourse import bacc, mybir

    sqa = float(np.sqrt(ALPHA))

    nc = bacc.Bacc("TRN2", target_bir_lowering=False, debug=False,
                   enable_asserts=True, num_devices=N_CORES)
    x_d = nc.dram_tensor("x", [P, FD], mybir.dt.float32, kind="ExternalInput").ap()
    y_d = nc.dram_tensor("y", [P, FD], mybir.dt.float32, kind="ExternalInput").ap()
    o_d = nc.dram_tensor("o", [P, 2 * N_GROUPS], mybir.dt.float32,
                         kind="ExternalOutput").ap()

    f32 = mybir.dt.float32
    bf16 = mybir.dt.bfloat16
    Sq = mybir.ActivationFunctionType.Square
    Alu = mybir.AluOpType

    with tile.TileContext(nc) as tc, ExitStack() as ctx:
        xp = ctx.enter_context(tc.tile_pool(name="x", bufs=BUFS[0]))
        yp = ctx.enter_context(tc.tile_pool(name="y", bufs=BUFS[1]))
        wp = ctx.enter_context(tc.tile_pool(name="work", bufs=BUFS[2]))
        sp = ctx.enter_context(tc.tile_pool(name="scratch", bufs=BUFS[3]))
        ap_ = ctx.enter_context(tc.tile_pool(name="acc", bufs=1))

        acc = ap_.tile([P, 2 * N_GROUPS], f32)

        deferred = []
        off = 0
        gdmx = gxm = None
        gdoff = gxoff = 0
        for t, cells in enumerate(TILE_SIZES):
            fd = cells * F
            g, k_in_g = _tile_group[t]
            gn = GROUP_OF[g]
            gcells = sum(TILE_SIZES[t - k_in_g:t - k_in_g + gn])
            xt = xp.tile([P, fd], f32, tag="xt")
            yt = yp.tile([P, fd], f32, tag="yt")
            sl = slice(off, off + fd)
            off += fd
            # two issue queues in lockstep (alternate whole tiles): one
            # queue's ~1300ns SEQ+HWDGE prep per pair can't keep up with
            # the 1.8us/tile stream, and it can't run ahead either
            q = nc.sync if t % 2 == 0 else nc.scalar
            q.dma_start(yt[:], y_d[:, sl])
            q.dma_start(xt[:], x_d[:, sl])

            if k_in_g == 0:
                # group buffers: dmx = [dm(t0)|xs0(t0)|dm(t1)|xs0(t1)|...]
                gdmx = wp.tile([P, (gcells * F) + gcells], bf16, tag="dmx")
                gxm = wp.tile([P, gcells * F], bf16, tag="xmg")
                gdoff = gxoff = 0

            # DVE: mask replicated to all 5 features (2x mode)
            m5 = wp.tile([P, fd], bf16, tag="m5")
            y0b = yt[:, 0::F].unsqueeze(2).broadcast_to((P, cells, F))
            nc.vector.tensor_scalar(
                m5[:].rearrange("p (k f) -> p k f", f=F), y0b,
                THRESH, None, op0=Alu.is_gt)

            # Pool: xs0 = sqrt(a)*x0 into this tile's dmx tail slice
            nc.gpsimd.tensor_scalar(
                gdmx[:, gdoff + fd:gdoff + fd + cells], xt[:, 0::F],
                sqa, None, op0=Alu.mult)

            # DVE: d = y - x (bf16 out), dm = d*m5 (bf16 2x)
            dt_ = wp.tile([P, fd], bf16, tag="d")
            nc.vector.tensor_tensor(dt_[:], yt[:], xt[:], op=Alu.subtract)
            nc.vector.tensor_tensor(gdmx[:, gdoff:gdoff + fd], dt_[:], m5[:],
                                    op=Alu.mult)
            xm_eng = nc.vector if t in XM_ON_DVE else nc.gpsimd
            xm_eng.tensor_tensor(gxm[:, gxoff:gxoff + fd], xt[:], m5[:],
                                 op=Alu.mult)
            gdoff += fd + cells
            gxoff += fd

            if k_in_g == gn - 1:
                n_dve_after = sum(1 for gg in range(g, N_GROUPS)
                                  if GROUP_ENG[gg] == 'dve')
                if GROUP_ENG[g] == 'act':
                    sq = sp.tile([P, gcells * F + gcells], bf16, tag="sq")
                    nc.scalar.activation(sq[:], gdmx[:], Sq,
                                         accum_out=acc[:, 2 * g:2 * g + 1])
                    sq2 = sp.tile([P, gcells * F], bf16, tag="sq2")
                    nc.scalar.activation(sq2[:], gxm[:], Sq,
                                         accum_out=acc[:, 2 * g + 1:2 * g + 2])
                elif n_dve_after <= DEFER_LAST:
                    deferred.append((g, gdmx, gxm, gcells))
                else:
                    _emit_ttr(nc, sp, acc, g, gdmx, gxm, gcells, bf16, Alu)

        for (g, gdmx, gxm, gcells) in deferred:
            _emit_ttr(nc, sp, acc, g, gdmx, gxm, gcells, bf16, Alu)

        nc.sync.dma_start(o_d[:], acc[:])

    nc.compile()
    return nc


def _emit_ttr(nc, sp, acc, g, gdmx, gxm, gcells, bf16, Alu):
    # DVE fused multiply + row-sum accumulate: acc += sum(in0*in1) per row
    scr = sp.tile([P, gcells * 6], bf16, tag="ttr1")
    nc.vector.scalar_tensor_tensor(
        scr[:], gdmx[:], 1.0, gdmx[:], op0=Alu.mult, op1=Alu.mult,
        accum_out=acc[:, 2 * g:2 * g + 1])
    scr2 = sp.tile([P, gcells * 5], bf16, tag="ttr2")
    nc.vector.scalar_tensor_tensor(
        scr2[:], gxm[:], 1.0, gxm[:], op0=Alu.mult, op1=Alu.mult,
        accum_out=acc[:, 2 * g + 1:2 * g + 2])


def _shard(a: np.ndarray) -> list[np.ndarray]:
    flat = a.reshape(-1)
    pad = np.zeros(PAD_CELLS * F, dtype=a.dtype)
    flat = np.concatenate([flat, pad])
    per_core = flat.reshape(N_CORES, P, FD)
    return [np.ascontiguousarray(per_core[i]) for i in range(N_CORES)]


def kernel(x: np.ndarray, y: np.ndarray) -> np.ndarray:
    global _compiled
    if _compiled is None:
        _compiled = _build()
    nc = _compiled

    from concourse.bass_utils import run_bass_kernel_spmd

    xs = _shard(np.asarray(x, dtype=np.float32))
    ys = _shard(np.asarray(y, dtype=np.float32))
    in_maps = [{"x": xs[i], "y": ys[i]} for i in range(N_CORES)]
    res = run_bass_kernel_spmd(nc, in_maps, core_ids=list(range(N_CORES)))

    total = np.float64(0.0)
    for r in res.results:
        o = r["o"].astype(np.float64).reshape(P, 2 * N_GROUPS)
        total += o[:, 0::2].sum()
        total -= ALPHA * o[:, 1::2].sum()
    return np.float32(total)


# revision 7
# speedup vs baseline: 1.1981x; 1.1320x over previous
"""Masked-loss kernel for nn_MLoss_9715216024200 on 8 Trainium2 NeuronCores.

loss = sum(where(y[...,0]>0.5, (y-x)^2 - a*x^2, 0)) + a*sum(x[...,0]^2)
with x,y f32 (256, 10647, 5); output is a f32 scalar.

Sharding: flatten both tensors to cells (5 contiguous f32 each), pad with
256 zero-cells (neutral: y0=0 -> mask 0, x=0 -> no bg term), reshape to
(8 cores, 128 partitions, 2662 cells).  Each core streams its 13 MiB at
the 360 GB/s DMA roofline (the stream is gapless); the design minimizes
the HEAD (fixed ~2us) and the TAIL (what happens after the last byte).

Per tile (cells ~134 shrinking to 8 at the end):
  DVE :  m5 = bf16(y0>0.5) broadcast to 5 features  (2x mode, 0.52ns/elem)
         d  = y - x (f32 1x, bf16 out)
         dm = d * m5 (bf16 2x) -> group dmx buffer [dm|xs0] slices
  Pool:  xs0 = sqrt(a)*x0 -> dmx tail slice;  xm = x*m5 -> group xm buffer
Reductions per GROUP of tiles (m*v^2 == (m*v)^2 since m is 0/1):
  acc[2g]   = sum(dmx^2) = sum((m*d)^2) + a*sum(x0^2)
  acc[2g+1] = sum(xm^2)            (host multiplies by -a)
Main groups use ACT Square+accum (ACT is otherwise idle); the small
suffix groups use DVE scalar_tensor_tensor accumulates so the endgame
after the final DMA semaphore is a short single-engine chain.

The tail tiles shrink gradually (96..8) so DVE's ~900ns semaphore lag
drains before the last tile; the final tile's whole chain (m5,d,dm,xm,
2 TTRs) runs on DVE back-to-back.  Host combines in f64:
total = sum(acc[even]) - a*sum(acc[odd]).
"""
import sys

for _p in ('/opt/trn_rl_repo',):
    if _p in sys.path:
        sys.path.remove(_p)
    sys.path.insert(0, _p)

import numpy as np

B, C, F = 256, 10647, 5
THRESH = 0.5
ALPHA = 0.1
N_CORES = 8
P = 128
CELLS = B * C                      # 2,725,632
CELLS_PER_PART = 2662              # 8*128*2662 = 2,725,888
PAD_CELLS = N_CORES * P * CELLS_PER_PART - CELLS   # 256
FD = CELLS_PER_PART * F            # 13310 elems per partition per core

# --- schedule configuration -------------------------------------------
SUFFIX = [96, 72, 56, 40, 28, 16, 8]          # shrinking tail tiles
_main_total = CELLS_PER_PART - sum(SUFFIX)    # 2346
_n_main = 18
_base = _main_total // _n_main
_rem = _main_total - _base * _n_main
MAIN = [_base + (1 if i < _rem else 0) for i in range(_n_main)]
TILE_SIZES = MAIN + SUFFIX
assert sum(TILE_SIZES) == CELLS_PER_PART

# reduction groups over consecutive tiles; engine 'act' or 'dve'
GROUP_OF = [3] * 6 + [2, 2, 2, 1]             # main 6x3, suffix {96,72},{56,40},{28,16},{8}
GROUP_ENG = ['act'] * 6 + ['act', 'act', 'dve', 'dve']
assert sum(GROUP_OF) == len(TILE_SIZES)
N_GROUPS = len(GROUP_OF)
_tile_group = [(gi, k) for gi, gn in enumerate(GROUP_OF) for k in range(gn)]
XM_ON_DVE = {len(TILE_SIZES) - 1}   # last tile: keep whole chain on DVE
DEFER_LAST = 1                      # defer the last k dve-group TTR pairs
BUFS = (12, 12, 8, 4)

_compiled = None


def _build():
    from contextlib import ExitStack
    import concourse.tile as tile
    from concourse import bacc, mybir

    sqa = float(np.sqrt(ALPHA))

    nc = bacc.Bacc("TRN2", target_bir_lowering=False, debug=False,
                   enable_asserts=True, num_devices=N_CORES)
    # host interleaves per tile: [y_tile | x_tile] so one DMA instruction
    # loads both (halves the ~700ns/DMA SP.SEQ+HWDGE issue cost)
    xy_d = nc.dram_tensor("xy", [P, 2 * FD], mybir.dt.float32,
                          kind="ExternalInput").ap()
    o_d = nc.dram_tensor("o", [P, 2 * N_GROUPS], mybir.dt.float32,
                         kind="ExternalOutput").ap()

    f32 = mybir.dt.float32
    bf16 = mybir.dt.bfloat16
    Sq = mybir.ActivationFunctionType.Square
    Alu = mybir.AluOpType

    with tile.TileContext(nc) as tc, ExitStack() as ctx:
        xyp = ctx.enter_context(tc.tile_pool(name="xy", bufs=BUFS[0]))
        wp = ctx.enter_context(tc.tile_pool(name="work", bufs=BUFS[2]))
        sp = ctx.enter_context(tc.tile_pool(name="scratch", bufs=BUFS[3]))
        ap_ = ctx.enter_context(tc.tile_pool(name="acc", bufs=1))

        acc = ap_.tile([P, 2 * N_GROUPS], f32)

        deferred = []
        off = 0
        gdmx = gxm = None
        gdoff = gxoff = 0
        for t, cells in enumerate(TILE_SIZES):
            fd = cells * F
            g, k_in_g = _tile_group[t]
            gn = GROUP_OF[g]
            gcells = sum(TILE_SIZES[t - k_in_g:t - k_in_g + gn])
            xyt = xyp.tile([P, 2 * fd], f32, tag="xyt")
            nc.sync.dma_start(xyt[:], xy_d[:, 2 * off:2 * off + 2 * fd])
            yt = xyt[:, :fd]
            xt = xyt[:, fd:]
            off += fd

            if k_in_g == 0:
                # group buffers: dmx = [dm(t0)|xs0(t0)|dm(t1)|xs0(t1)|...]
                gdmx = wp.tile([P, (gcells * F) + gcells], bf16, tag="dmx")
                gxm = wp.tile([P, gcells * F], bf16, tag="xmg")
                gdoff = gxoff = 0

            # DVE: mask replicated to all 5 features (2x mode)
            m5 = wp.tile([P, fd], bf16, tag="m5")
            y0b = yt[:, 0::F].unsqueeze(2).broadcast_to((P, cells, F))
            nc.vector.tensor_scalar(
                m5[:].rearrange("p (k f) -> p k f", f=F), y0b,
                THRESH, None, op0=Alu.is_gt)

            # Pool: xs0 = sqrt(a)*x0 into this tile's dmx tail slice
            nc.gpsimd.tensor_scalar(
                gdmx[:, gdoff + fd:gdoff + fd + cells], xt[:, 0::F],
                sqa, None, op0=Alu.mult)

            # DVE: d = y - x (bf16 out), dm = d*m5 (bf16 2x)
            dt_ = wp.tile([P, fd], bf16, tag="d")
            nc.vector.tensor_tensor(dt_[:], yt, xt, op=Alu.subtract)
            nc.vector.tensor_tensor(gdmx[:, gdoff:gdoff + fd], dt_[:], m5[:],
                                    op=Alu.mult)
            xm_eng = nc.vector if t in XM_ON_DVE else nc.gpsimd
            xm_eng.tensor_tensor(gxm[:, gxoff:gxoff + fd], xt, m5[:],
                                 op=Alu.mult)
            gdoff += fd + cells
            gxoff += fd

            if k_in_g == gn - 1:
                n_dve_after = sum(1 for gg in range(g, N_GROUPS)
                                  if GROUP_ENG[gg] == 'dve')
                if GROUP_ENG[g] == 'act':
                    sq = sp.tile([P, gcells * F + gcells], bf16, tag="sq")
                    nc.scalar.activation(sq[:], gdmx[:], Sq,
                                         accum_out=acc[:, 2 * g:2 * g + 1])
                    sq2 = sp.tile([P, gcells * F], bf16, tag="sq2")
                    nc.scalar.activation(sq2[:], gxm[:], Sq,
                                         accum_out=acc[:, 2 * g + 1:2 * g + 2])
                elif n_dve_after <= DEFER_LAST:
                    deferred.append((g, gdmx, gxm, gcells))
                else:
                    _emit_ttr(nc, sp, acc, g, gdmx, gxm, gcells, bf16, Alu)

        for (g, gdmx, gxm, gcells) in deferred:
            _emit_ttr(nc, sp, acc, g, gdmx, gxm, gcells, bf16, Alu)

        nc.sync.dma_start(o_d[:], acc[:])

    nc.compile()
    return nc


def _emit_ttr(nc, sp, acc, g, gdmx, gxm, gcells, bf16, Alu):
    # DVE fused multiply + row-sum accumulate: acc += sum(in0*in1) per row
    scr = sp.tile([P, gcells * 6], bf16, tag="ttr1")
    nc.vector.scalar_tensor_tensor(
        scr[:], gdmx[:], 1.0, gdmx[:], op0=Alu.mult, op1=Alu.mult,
        accum_out=acc[:, 2 * g:2 * g + 1])
    scr2 = sp.tile([P, gcells * 5], bf16, tag="ttr2")
    nc.vector.scalar_tensor_tensor(
        scr2[:], gxm[:], 1.0, gxm[:], op0=Alu.mult, op1=Alu.mult,
        accum_out=acc[:, 2 * g + 1:2 * g + 2])


def _shard_xy(x: np.ndarray, y: np.ndarray) -> list[np.ndarray]:
    """Per core: [P, 2*FD] with per-tile interleave [y_tile | x_tile]."""
    pad = np.zeros(PAD_CELLS * F, dtype=np.float32)
    xf = np.concatenate([x.reshape(-1), pad]).reshape(N_CORES, P, FD)
    yf = np.concatenate([y.reshape(-1), pad]).reshape(N_CORES, P, FD)
    out = np.empty((N_CORES, P, 2 * FD), dtype=np.float32)
    off = 0
    for cells in TILE_SIZES:
        fd = cells * F
        out[:, :, 2 * off:2 * off + fd] = yf[:, :, off:off + fd]
        out[:, :, 2 * off + fd:2 * off + 2 * fd] = xf[:, :, off:off + fd]
        off += fd
    return [np.ascontiguousarray(out[i]) for i in range(N_CORES)]


def kernel(x: np.ndarray, y: np.ndarray) -> np.ndarray:
    global _compiled
    if _compiled is None:
        _compiled = _build()
    nc = _compiled

    from concourse.bass_utils import run_bass_kernel_spmd

    xys = _shard_xy(np.asarray(x, dtype=np.float32),
                    np.asarray(y, dtype=np.float32))
    in_maps = [{"xy": xys[i]} for i in range(N_CORES)]
    res = run_bass_kernel_spmd(nc, in_maps, core_ids=list(range(N_CORES)))

    total = np.float64(0.0)
    for r in res.results:
        o = r["o"].astype(np.float64).reshape(P, 2 * N_GROUPS)
        total += o[:, 0::2].sum()
        total -= ALPHA * o[:, 1::2].sum()
    return np.float32(total)
